# revision 9
# baseline (speedup 1.0000x reference)
"""Single-step LSTM cell (B=131072, E=H=128) on 8 Trainium2 NeuronCores.

Strategy: pure data-parallel over the batch; each core handles 16384 rows
in transposed layout (contraction dim on SBUF partitions, batch on the
free dim). Work is organized gate-major over variable-width segments
(small at head and tail so the first ACTIVATE issues early and the
post-matmul drain chain is short), grouped into supersegments for DMA
and DVE-polynomial granularity. x and h are packed into one DRAM tensor
so each segment needs a single input DMA (3D access pattern); c rides a
separate per-supersegment DMA. Each gate gets a [128, w] fp32 PSUM slice
(double buffered), filled by 512-column bf16 matmuls (W then U), then
drained by a per-gate ACTIVATE whose per-partition bias operand applies
the gate bias for free. The c path is all bf16 (budget 2e-2, ~1.2e-2
achieved). ScalarE is the binding engine (4 gate evals per element), so
tanh(c) mostly runs on the DVE as a clamped degree-5 odd polynomial in
completed-square form using only 2x tensor_tensor and 4x tensor_scalar
ops; two middle segments defer tanh(c) to the ACT queue to balance the
engines. A dummy no-bias ACTIVATE right after the boot barrier
prefetches the sigmoid/tanh table set off the critical path, and warmup
matmuls ramp the PE clock (HAM) before the first real MMs.
"""

import numpy as np

B, E, H = 131072, 128, 128
NCORES = 8
BC = B // NCORES        # 16384 batch rows per core
S = 512                 # matmul moving cols (one PSUM bank)
CW = 4096               # chunk stride inside packed xh SBUF tiles

SEGS = [512, 1536, 2048, 2048, 2048, 2048, 2048, 2048, 1536, 512]
assert sum(SEGS) == BC
# supersegments (start_seg, end_seg): DMA + poly granularity
SUPER = [(0, 2), (2, 4), (4, 6), (6, 8), (8, 9), (9, 10)]
# segments whose tanh(c) runs on the ACT queue deferred into the next
# supersegment's gate stream (must be the last segment of their ss)
ACT_TANH = {3, 5}
# tail segments whose tanh(c) runs on ACT immediately (keeps the DVE
# polynomial chain off the drain path at the end of the kernel)
ACT_INLINE = {8, 9}
# segments whose m1 = f*c_prev runs on GPSIMD instead of the DVE
GP_M1 = set(range(10))

_CACHE = {}


def _build_nc():
    import concourse.bacc as bacc
    import concourse.mybir as mybir
    import concourse.tile as tile

    f32 = mybir.dt.float32
    bf = mybir.dt.bfloat16
    AF = mybir.ActivationFunctionType
    ALU = mybir.AluOpType

    nc = bacc.Bacc("TRN2", target_bir_lowering=False, debug=False,
                   num_devices=NCORES)

    xh = nc.dram_tensor("xh", [E, 2 * BC], bf, kind="ExternalInput").ap()
    cT = nc.dram_tensor("cT", [H, BC], bf, kind="ExternalInput").ap()
    W = nc.dram_tensor("W", [E, 4 * H], bf, kind="ExternalInput").ap()
    U = nc.dram_tensor("U", [H, 4 * H], bf, kind="ExternalInput").ap()
    bias = nc.dram_tensor("b", [H, 4], f32, kind="ExternalInput").ap()
    hT_out = nc.dram_tensor("hT_out", [H, BC], bf, kind="ExternalOutput").ap()
    cT_out = nc.dram_tensor("cT_out", [H, BC], bf, kind="ExternalOutput").ap()

    xh3 = xh.rearrange("p (c n) -> p c n", c=2)

    NSEG = len(SEGS)
    OFF = [sum(SEGS[:i]) for i in range(NSEG)]
    NSS = len(SUPER)
    SSOFF = [OFF[a] for a, _ in SUPER]
    SSW = [sum(SEGS[a:b]) for a, b in SUPER]

    # tanh(x) ~ ((SQB2*x^2 - ASQB2)^2 + B2D) * x on [-PR, PR]
    SQB2 = 0.11248462
    ASQB2 = 0.74192809
    B2D = 0.37794151
    PR = 3.0

    with tile.TileContext(nc) as tc:
        with tc.tile_pool(name="cst", bufs=1) as cst, \
             tc.tile_pool(name="xin", bufs=3) as xin, \
             tc.tile_pool(name="cin", bufs=3) as cin, \
             tc.tile_pool(name="ga", bufs=2) as gap, \
             tc.tile_pool(name="tcp", bufs=2) as tcp, \
             tc.tile_pool(name="mw", bufs=1) as mw, \
             tc.tile_pool(name="pp", bufs=1) as pp, \
             tc.tile_pool(name="co", bufs=2) as cop, \
             tc.tile_pool(name="ho", bufs=2) as hop, \
             tc.tile_pool(name="ps", bufs=2, space="PSUM") as ps:

            W_sb = cst.tile([E, 4 * H], bf)
            U_sb = cst.tile([H, 4 * H], bf)
            b_sb = cst.tile([H, 4], f32)

            xh_t = [None] * NSS    # packed [x | h] tile per supersegment
            c_t = [None] * NSS

            def alloc_in(ss):
                xh_t[ss] = xin.tile([E, 2 * CW], bf, tag="xh", name=f"xh{ss}")
                c_t[ss] = cin.tile([H, CW], bf, tag="c", name=f"c{ss}")

            def dma_xh_seg(ss, k):
                # one fused [x|h] DMA per segment (3D access pattern)
                off, w = OFF[k], SEGS[k]
                loc = off - SSOFF[ss]
                dst = xh_t[ss].rearrange("p (c n) -> p c n", n=CW)
                nc.sync.dma_start(out=dst[:, :, loc:loc + w],
                                  in_=xh3[:, :, off:off + w])

            def dma_c(ss):
                off, w = SSOFF[ss], SSW[ss]
                nc.sync.dma_start(out=c_t[ss][:, :w], in_=cT[:, off:off + w])

            def prefetch(ss):
                if ss < NSS:
                    alloc_in(ss)
                    for k in range(*SUPER[ss]):
                        dma_xh_seg(ss, k)
                    dma_c(ss)

            # head: W, b, seg0's x/h, U, then seg1, c(ss0), ss1's x/h, c(ss1)
            nc.sync.dma_start(out=W_sb[:], in_=W)
            nc.sync.dma_start(out=b_sb[:], in_=bias)
            alloc_in(0)
            dma_xh_seg(0, 0)
            nc.sync.dma_start(out=U_sb[:], in_=U)
            dma_xh_seg(0, 1)
            dma_c(0)
            alloc_in(1)
            dma_xh_seg(1, 2)
            dma_xh_seg(1, 3)
            dma_c(1)

            # warm the PE (HAM ramp) + prefetch the sigmoid/tanh ACT table
            wsrc = cst.tile([E, S], bf, name="wsrc")
            nc.gpsimd.memset(wsrc[:], 1.0)
            tbl = cst.tile([H, 8], bf, name="tbl")
            nc.scalar.activation(tbl[:], wsrc[:, 0:8], AF.Sigmoid)
            warm = ps.tile([H, 2048], f32, tag="g")
            for _ in range(6):
                nc.tensor.matmul(warm[:, 0:S], wsrc[:, 0:H], wsrc[:],
                                 start=True, stop=True)

            # gate order: 0=i, 1=c~, 2=f, 3=o
            pend = None  # (a3, co_tile, ho_tile, off, w, loc)

            for ssi in range(NSS):
                prefetch(ssi + 2)
                a, bnd = SUPER[ssi]
                ssoff, ssw = SSOFF[ssi], SSW[ssi]
                xh_sb, c_sb = xh_t[ssi], c_t[ssi]
                co_sb = cop.tile([H, CW], bf, tag="co", name=f"co{ssi}")
                ho_sb = hop.tile([H, CW], bf, tag="ho", name=f"ho{ssi}")
                o_gates = {}
                my_pend = None

                for k in range(a, bnd):
                    off, w = OFF[k], SEGS[k]
                    loc = off - ssoff
                    ns = w // S
                    x_sl = xh_sb[:, loc:loc + w]
                    h_sl = xh_sb[:, CW + loc:CW + loc + w]
                    gates = [None] * 4
                    for g in range(4):
                        gp = ps.tile([H, 2048], f32, tag="g")
                        Wg = W_sb[:, g * H:(g + 1) * H]
                        Ug = U_sb[:, g * H:(g + 1) * H]
                        for s in range(ns):
                            sl = slice(s * S, (s + 1) * S)
                            nc.tensor.matmul(gp[:, sl], Wg, x_sl[:, sl],
                                             start=True, stop=False)
                        for s in range(ns):
                            sl = slice(s * S, (s + 1) * S)
                            nc.tensor.matmul(gp[:, sl], Ug, h_sl[:, sl],
                                             start=False, stop=True)
                        ab = gap.tile([H, 2048], bf, tag=f"a{g}",
                                      bufs=3 if g == 3 else 2,
                                      name=f"a{g}_{k}")
                        func = AF.Tanh if g == 1 else AF.Sigmoid
                        nc.scalar.activation(ab[:, :w], gp[:, :w], func,
                                             bias=b_sb[:, g:g + 1])
                        gates[g] = ab
                        if g == 2 and k == a and pend is not None:
                            # tanh(c) of the previous ACT_TANH segment,
                            # slotted into the ACT queue mid-gate-stream
                            pw = pend[4]
                            t_prev = tcp.tile([H, 2048], bf, tag="t")
                            nc.scalar.activation(
                                t_prev[:, :pw],
                                pend[1][:, pend[5]:pend[5] + pw], AF.Tanh)

                    # DVE: c = f*c_prev + i*c~  (per segment, all bf16 2x)
                    m2 = mw.tile([H, 2048], bf, tag="m2", name=f"m2_{k}")
                    nc.vector.tensor_mul(out=m2[:, :w], in0=gates[0][:, :w],
                                         in1=gates[1][:, :w])
                    m1 = mw.tile([H, 2048], bf, tag="m1", name=f"m1_{k}")
                    eng = nc.gpsimd if k in GP_M1 else nc.vector
                    eng.tensor_mul(out=m1[:, :w], in0=gates[2][:, :w],
                                   in1=c_sb[:, loc:loc + w])
                    nc.vector.tensor_add(out=co_sb[:, loc:loc + w],
                                         in0=m1[:, :w], in1=m2[:, :w])
                    o_gates[k] = gates[3]

                    if k in ACT_TANH:
                        assert k == bnd - 1
                        my_pend = (gates[3], co_sb, ho_sb, off, w, loc)
                    elif k in ACT_INLINE:
                        # tail: tanh(c) on ACT right away, short drain chain
                        t_in = tcp.tile([H, 2048], bf, tag="t",
                                        name=f"ti{k}")
                        nc.scalar.activation(t_in[:, :w],
                                             co_sb[:, loc:loc + w], AF.Tanh)
                        nc.vector.tensor_mul(out=ho_sb[:, loc:loc + w],
                                             in0=gates[3][:, :w],
                                             in1=t_in[:, :w])
                        nc.sync.dma_start(out=hT_out[:, off:off + w],
                                          in_=ho_sb[:, loc:loc + w])

                # resolve previous supersegment's deferred tanh(c)
                if pend is not None:
                    pa3, pco, pho, poff, pw, ploc = pend
                    nc.vector.tensor_mul(out=pho[:, ploc:ploc + pw],
                                         in0=pa3[:, :pw], in1=t_prev[:, :pw])
                    nc.sync.dma_start(out=hT_out[:, poff:poff + pw],
                                      in_=pho[:, ploc:ploc + pw])
                    pend = None

                # c out for the whole supersegment
                nc.sync.dma_start(out=cT_out[:, ssoff:ssoff + ssw],
                                  in_=co_sb[:, :ssw])

                # tanh(c) polynomial on the DVE over the poly-portion
                pwid = sum(SEGS[k] for k in range(a, bnd)
                           if k not in ACT_TANH and k not in ACT_INLINE)
                if pwid > 0:
                    xc = pp.tile([H, CW], bf, tag="pc")
                    nc.vector.tensor_scalar(out=xc[:, :pwid],
                                            in0=co_sb[:, :pwid],
                                            scalar1=PR, scalar2=-PR,
                                            op0=ALU.min, op1=ALU.max)
                    u = pp.tile([H, CW], bf, tag="pA", name=f"u{ssi}")
                    nc.vector.tensor_mul(out=u[:, :pwid], in0=xc[:, :pwid],
                                         in1=xc[:, :pwid])
                    wp = pp.tile([H, CW], bf, tag="pB", name=f"w{ssi}")
                    nc.vector.tensor_scalar(out=wp[:, :pwid], in0=u[:, :pwid],
                                            scalar1=SQB2, scalar2=ASQB2,
                                            op0=ALU.mult, op1=ALU.subtract)
                    v = pp.tile([H, CW], bf, tag="pA", name=f"v{ssi}")
                    nc.vector.tensor_mul(out=v[:, :pwid], in0=wp[:, :pwid],
                                         in1=wp[:, :pwid])
                    y2 = pp.tile([H, CW], bf, tag="pB", name=f"y2{ssi}")
                    nc.vector.tensor_scalar(out=y2[:, :pwid],
                                            in0=v[:, :pwid],
                                            scalar1=B2D, scalar2=None,
                                            op0=ALU.add)
                    ty = pp.tile([H, CW], bf, tag="pA", name=f"ty{ssi}")
                    nc.vector.tensor_mul(out=ty[:, :pwid], in0=y2[:, :pwid],
                                         in1=xc[:, :pwid])
                    # h = o * tanh(c), per segment (o tiles are per-seg)
                    for k in range(a, bnd):
                        off, w = OFF[k], SEGS[k]
                        loc = off - ssoff
                        if loc >= pwid:
                            continue
                        nc.vector.tensor_mul(
                            out=ho_sb[:, loc:loc + w],
                            in0=o_gates[k][:, :w],
                            in1=ty[:, loc:loc + w])
                    nc.sync.dma_start(out=hT_out[:, ssoff:ssoff + pwid],
                                      in_=ho_sb[:, :pwid])

                pend = my_pend

            assert pend is None

    nc.compile()
    return nc


def kernel(x, hidden_memory_tm1, Wi, Ui, bi, Wf, Uf, bf, Wog, Uog, bog,
           Wc, Uc, bc, _return_timing=False, _trace=False):
    from concourse.bass_utils import run_bass_kernel_spmd

    if "nc" not in _CACHE:
        _CACHE["nc"] = _build_nc()
    nc = _CACHE["nc"]

    import ml_dtypes
    bf16 = ml_dtypes.bfloat16
    x = np.asarray(x, np.float32)
    hm = np.asarray(hidden_memory_tm1, np.float32)
    # gate order i, c~, f, o (c~ second so the DVE can start i*c~ early)
    W = np.concatenate([Wi, Wc, Wf, Wog], axis=1).astype(bf16)
    U = np.concatenate([Ui, Uc, Uf, Uog], axis=1).astype(bf16)
    bcat = np.stack([np.asarray(bi), np.asarray(bc), np.asarray(bf),
                     np.asarray(bog)], axis=1).astype(np.float32)  # [H, 4]

    in_maps = []
    for c in range(NCORES):
        sl = slice(c * BC, (c + 1) * BC)
        xTc = np.ascontiguousarray(x[sl].astype(bf16).T)
        hTc = np.ascontiguousarray(hm[0, sl].astype(bf16).T)
        in_maps.append({
            "xh": np.ascontiguousarray(np.concatenate([xTc, hTc], axis=1)),
            "cT": np.ascontiguousarray(hm[1, sl].astype(bf16).T),
            "W": W, "U": U, "b": bcat,
        })

    res = run_bass_kernel_spmd(nc, in_maps, core_ids=list(range(NCORES)),
                               trace=_trace)

    h = np.concatenate(
        [res.results[c]["hT_out"].T.astype(np.float32)
         for c in range(NCORES)], 0)
    cc = np.concatenate(
        [res.results[c]["cT_out"].T.astype(np.float32)
         for c in range(NCORES)], 0)
    out = np.stack([h, cc])
    if _return_timing:
        return out, res
    return out


# revision 10
# speedup vs baseline: 1.2206x; 1.2206x over previous
"""Single-step LSTM cell (B=131072, E=H=128) on 8 Trainium2 NeuronCores.

Strategy: pure data-parallel over the batch; each core handles 16384 rows
in transposed layout (contraction dim on SBUF partitions, batch on the
free dim). Work is organized gate-major over variable-width segments
(small at head and tail so the first ACTIVATE issues early and the
post-matmul drain chain is short), grouped into supersegments for DMA
and DVE-polynomial granularity. x and h are packed into one DRAM tensor
so each segment needs a single input DMA (3D access pattern); c rides a
separate per-supersegment DMA. Each gate gets a [128, w] fp32 PSUM slice
(double buffered), filled by 512-column bf16 matmuls (W then U), then
drained by a per-gate ACTIVATE whose per-partition bias operand applies
the gate bias for free. The c path is all bf16 (budget 2e-2, ~1.2e-2
achieved). ScalarE is the binding engine (4 gate evals per element), so
tanh(c) mostly runs on the DVE as a clamped degree-5 odd polynomial in
completed-square form using only 2x tensor_tensor and 4x tensor_scalar
ops; two middle segments defer tanh(c) to the ACT queue to balance the
engines. A dummy no-bias ACTIVATE right after the boot barrier
prefetches the sigmoid/tanh table set off the critical path, and warmup
matmuls ramp the PE clock (HAM) before the first real MMs.
"""

import numpy as np

B, E, H = 131072, 128, 128
NCORES = 8
BC = B // NCORES        # 16384 batch rows per core
S = 512                 # matmul moving cols (one PSUM bank)
CW = 4096               # chunk stride inside packed xh SBUF tiles

SEGS = [512, 1536, 2048, 2048, 2048, 2048, 2048, 2048, 1536, 512]
assert sum(SEGS) == BC
# supersegments (start_seg, end_seg): DMA + poly granularity
SUPER = [(0, 2), (2, 4), (4, 6), (6, 8), (8, 9), (9, 10)]
# segments whose tanh(c) runs on the ACT queue deferred into the next
# supersegment's gate stream (must be the last segment of their ss)
ACT_TANH = {3, 5}
# tail segments whose tanh(c) runs on ACT immediately (keeps the DVE
# polynomial chain off the drain path at the end of the kernel)
ACT_INLINE = {8, 9}
# segments whose m1 = f*c_prev runs on GPSIMD instead of the DVE
GP_M1 = set()

_CACHE = {}


def _build_nc():
    import concourse.bacc as bacc
    import concourse.mybir as mybir
    import concourse.tile as tile

    f32 = mybir.dt.float32
    bf = mybir.dt.bfloat16
    AF = mybir.ActivationFunctionType
    ALU = mybir.AluOpType

    nc = bacc.Bacc("TRN2", target_bir_lowering=False, debug=False,
                   num_devices=NCORES)

    xh = nc.dram_tensor("xh", [E, 2 * BC], bf, kind="ExternalInput").ap()
    cT = nc.dram_tensor("cT", [H, BC], bf, kind="ExternalInput").ap()
    W = nc.dram_tensor("W", [E, 4 * H], bf, kind="ExternalInput").ap()
    U = nc.dram_tensor("U", [H, 4 * H], bf, kind="ExternalInput").ap()
    bias = nc.dram_tensor("b", [H, 4], f32, kind="ExternalInput").ap()
    hT_out = nc.dram_tensor("hT_out", [H, BC], bf, kind="ExternalOutput").ap()
    cT_out = nc.dram_tensor("cT_out", [H, BC], bf, kind="ExternalOutput").ap()

    xh3 = xh.rearrange("p (c n) -> p c n", c=2)

    NSEG = len(SEGS)
    OFF = [sum(SEGS[:i]) for i in range(NSEG)]
    NSS = len(SUPER)
    SSOFF = [OFF[a] for a, _ in SUPER]
    SSW = [sum(SEGS[a:b]) for a, b in SUPER]

    # tanh(x) ~ ((SQB2*x^2 - ASQB2)^2 + B2D) * x on [-PR, PR]
    SQB2 = 0.11248462
    ASQB2 = 0.74192809
    B2D = 0.37794151
    PR = 3.0

    with tile.TileContext(nc) as tc:
        with tc.tile_pool(name="cst", bufs=1) as cst, \
             tc.tile_pool(name="xin", bufs=3) as xin, \
             tc.tile_pool(name="cin", bufs=3) as cin, \
             tc.tile_pool(name="ga", bufs=2) as gap, \
             tc.tile_pool(name="tcp", bufs=2) as tcp, \
             tc.tile_pool(name="mw", bufs=1) as mw, \
             tc.tile_pool(name="pp", bufs=1) as pp, \
             tc.tile_pool(name="co", bufs=2) as cop, \
             tc.tile_pool(name="ho", bufs=2) as hop, \
             tc.tile_pool(name="ps", bufs=2, space="PSUM") as ps:

            W_sb = cst.tile([E, 4 * H], bf)
            U_sb = cst.tile([H, 4 * H], bf)
            b_sb = cst.tile([H, 4], f32)

            xh_t = [None] * NSS    # packed [x | h] tile per supersegment
            c_t = [None] * NSS

            def alloc_in(ss):
                xh_t[ss] = xin.tile([E, 2 * CW], bf, tag="xh", name=f"xh{ss}")
                c_t[ss] = cin.tile([H, CW], bf, tag="c", name=f"c{ss}")

            def dma_xh_seg(ss, k):
                # one fused [x|h] DMA per segment (3D access pattern)
                off, w = OFF[k], SEGS[k]
                loc = off - SSOFF[ss]
                dst = xh_t[ss].rearrange("p (c n) -> p c n", n=CW)
                nc.sync.dma_start(out=dst[:, :, loc:loc + w],
                                  in_=xh3[:, :, off:off + w])

            def dma_c(ss):
                off, w = SSOFF[ss], SSW[ss]
                nc.sync.dma_start(out=c_t[ss][:, :w], in_=cT[:, off:off + w])

            def prefetch(ss):
                if ss < NSS:
                    alloc_in(ss)
                    for k in range(*SUPER[ss]):
                        dma_xh_seg(ss, k)
                    dma_c(ss)

            # head: W, b, seg0's x/h, U, then seg1, c(ss0), ss1's x/h, c(ss1)
            nc.sync.dma_start(out=W_sb[:], in_=W)
            nc.sync.dma_start(out=b_sb[:], in_=bias)
            alloc_in(0)
            dma_xh_seg(0, 0)
            nc.sync.dma_start(out=U_sb[:], in_=U)
            dma_xh_seg(0, 1)
            dma_c(0)
            alloc_in(1)
            dma_xh_seg(1, 2)
            dma_xh_seg(1, 3)
            dma_c(1)

            # warm the PE (HAM ramp) + prefetch the sigmoid/tanh ACT table
            wsrc = cst.tile([E, S], bf, name="wsrc")
            nc.gpsimd.memset(wsrc[:], 1.0)
            tbl = cst.tile([H, 8], bf, name="tbl")
            nc.scalar.activation(tbl[:], wsrc[:, 0:8], AF.Sigmoid)
            warm = ps.tile([H, 2048], f32, tag="g")
            for _ in range(6):
                nc.tensor.matmul(warm[:, 0:S], wsrc[:, 0:H], wsrc[:],
                                 start=True, stop=True)

            # gate order: 0=i, 1=c~, 2=f, 3=o
            pend = None  # (a3, co_tile, ho_tile, off, w, loc)

            for ssi in range(NSS):
                prefetch(ssi + 2)
                a, bnd = SUPER[ssi]
                ssoff, ssw = SSOFF[ssi], SSW[ssi]
                xh_sb, c_sb = xh_t[ssi], c_t[ssi]
                co_sb = cop.tile([H, CW], bf, tag="co", name=f"co{ssi}")
                ho_sb = hop.tile([H, CW], bf, tag="ho", name=f"ho{ssi}")
                o_gates = {}
                my_pend = None

                for k in range(a, bnd):
                    off, w = OFF[k], SEGS[k]
                    loc = off - ssoff
                    ns = w // S
                    x_sl = xh_sb[:, loc:loc + w]
                    h_sl = xh_sb[:, CW + loc:CW + loc + w]
                    gates = [None] * 4
                    for g in range(4):
                        gp = ps.tile([H, 2048], f32, tag="g")
                        Wg = W_sb[:, g * H:(g + 1) * H]
                        Ug = U_sb[:, g * H:(g + 1) * H]
                        for s in range(ns):
                            sl = slice(s * S, (s + 1) * S)
                            nc.tensor.matmul(gp[:, sl], Wg, x_sl[:, sl],
                                             start=True, stop=False)
                        for s in range(ns):
                            sl = slice(s * S, (s + 1) * S)
                            nc.tensor.matmul(gp[:, sl], Ug, h_sl[:, sl],
                                             start=False, stop=True)
                        ab = gap.tile([H, 2048], bf, tag=f"a{g}",
                                      bufs=3 if g == 3 else 2,
                                      name=f"a{g}_{k}")
                        func = AF.Tanh if g == 1 else AF.Sigmoid
                        nc.scalar.activation(ab[:, :w], gp[:, :w], func,
                                             bias=b_sb[:, g:g + 1])
                        gates[g] = ab
                        if g == 2 and k == a and pend is not None:
                            # tanh(c) of the previous ACT_TANH segment,
                            # slotted into the ACT queue mid-gate-stream
                            pw = pend[4]
                            t_prev = tcp.tile([H, 2048], bf, tag="t")
                            nc.scalar.activation(
                                t_prev[:, :pw],
                                pend[1][:, pend[5]:pend[5] + pw], AF.Tanh)

                    # DVE: c = f*c_prev + i*c~  (per segment, all bf16 2x)
                    m2 = mw.tile([H, 2048], bf, tag="m2", name=f"m2_{k}")
                    nc.vector.tensor_mul(out=m2[:, :w], in0=gates[0][:, :w],
                                         in1=gates[1][:, :w])
                    m1 = mw.tile([H, 2048], bf, tag="m1", name=f"m1_{k}")
                    eng = nc.gpsimd if k in GP_M1 else nc.vector
                    eng.tensor_mul(out=m1[:, :w], in0=gates[2][:, :w],
                                   in1=c_sb[:, loc:loc + w])
                    nc.vector.tensor_add(out=co_sb[:, loc:loc + w],
                                         in0=m1[:, :w], in1=m2[:, :w])
                    o_gates[k] = gates[3]

                    if k in ACT_TANH:
                        assert k == bnd - 1
                        my_pend = (gates[3], co_sb, ho_sb, off, w, loc)
                    elif k in ACT_INLINE:
                        # tail: tanh(c) on ACT right away, short drain chain
                        t_in = tcp.tile([H, 2048], bf, tag="t",
                                        name=f"ti{k}")
                        nc.scalar.activation(t_in[:, :w],
                                             co_sb[:, loc:loc + w], AF.Tanh)
                        nc.vector.tensor_mul(out=ho_sb[:, loc:loc + w],
                                             in0=gates[3][:, :w],
                                             in1=t_in[:, :w])
                        nc.sync.dma_start(out=hT_out[:, off:off + w],
                                          in_=ho_sb[:, loc:loc + w])

                # resolve previous supersegment's deferred tanh(c)
                if pend is not None:
                    pa3, pco, pho, poff, pw, ploc = pend
                    nc.vector.tensor_mul(out=pho[:, ploc:ploc + pw],
                                         in0=pa3[:, :pw], in1=t_prev[:, :pw])
                    nc.sync.dma_start(out=hT_out[:, poff:poff + pw],
                                      in_=pho[:, ploc:ploc + pw])
                    pend = None

                # c out for the whole supersegment
                nc.sync.dma_start(out=cT_out[:, ssoff:ssoff + ssw],
                                  in_=co_sb[:, :ssw])

                # tanh(c) polynomial on the DVE over the poly-portion
                pwid = sum(SEGS[k] for k in range(a, bnd)
                           if k not in ACT_TANH and k not in ACT_INLINE)
                if pwid > 0:
                    xc = pp.tile([H, CW], bf, tag="pc")
                    nc.vector.tensor_scalar(out=xc[:, :pwid],
                                            in0=co_sb[:, :pwid],
                                            scalar1=PR, scalar2=-PR,
                                            op0=ALU.min, op1=ALU.max)
                    u = pp.tile([H, CW], bf, tag="pA", name=f"u{ssi}")
                    nc.vector.tensor_mul(out=u[:, :pwid], in0=xc[:, :pwid],
                                         in1=xc[:, :pwid])
                    wp = pp.tile([H, CW], bf, tag="pB", name=f"w{ssi}")
                    nc.vector.tensor_scalar(out=wp[:, :pwid], in0=u[:, :pwid],
                                            scalar1=SQB2, scalar2=ASQB2,
                                            op0=ALU.mult, op1=ALU.subtract)
                    v = pp.tile([H, CW], bf, tag="pA", name=f"v{ssi}")
                    nc.vector.tensor_mul(out=v[:, :pwid], in0=wp[:, :pwid],
                                         in1=wp[:, :pwid])
                    y2 = pp.tile([H, CW], bf, tag="pB", name=f"y2{ssi}")
                    nc.vector.tensor_scalar(out=y2[:, :pwid],
                                            in0=v[:, :pwid],
                                            scalar1=B2D, scalar2=None,
                                            op0=ALU.add)
                    ty = pp.tile([H, CW], bf, tag="pA", name=f"ty{ssi}")
                    nc.vector.tensor_mul(out=ty[:, :pwid], in0=y2[:, :pwid],
                                         in1=xc[:, :pwid])
                    # h = o * tanh(c), per segment (o tiles are per-seg)
                    for k in range(a, bnd):
                        off, w = OFF[k], SEGS[k]
                        loc = off - ssoff
                        if loc >= pwid:
                            continue
                        nc.vector.tensor_mul(
                            out=ho_sb[:, loc:loc + w],
                            in0=o_gates[k][:, :w],
                            in1=ty[:, loc:loc + w])
                    nc.sync.dma_start(out=hT_out[:, ssoff:ssoff + pwid],
                                      in_=ho_sb[:, :pwid])

                pend = my_pend

            assert pend is None

    nc.compile()
    return nc


def kernel(x, hidden_memory_tm1, Wi, Ui, bi, Wf, Uf, bf, Wog, Uog, bog,
           Wc, Uc, bc, _return_timing=False, _trace=False):
    from concourse.bass_utils import run_bass_kernel_spmd

    if "nc" not in _CACHE:
        _CACHE["nc"] = _build_nc()
    nc = _CACHE["nc"]

    import ml_dtypes
    bf16 = ml_dtypes.bfloat16
    x = np.asarray(x, np.float32)
    hm = np.asarray(hidden_memory_tm1, np.float32)
    # gate order i, c~, f, o (c~ second so the DVE can start i*c~ early)
    W = np.concatenate([Wi, Wc, Wf, Wog], axis=1).astype(bf16)
    U = np.concatenate([Ui, Uc, Uf, Uog], axis=1).astype(bf16)
    bcat = np.stack([np.asarray(bi), np.asarray(bc), np.asarray(bf),
                     np.asarray(bog)], axis=1).astype(np.float32)  # [H, 4]

    in_maps = []
    for c in range(NCORES):
        sl = slice(c * BC, (c + 1) * BC)
        xTc = np.ascontiguousarray(x[sl].astype(bf16).T)
        hTc = np.ascontiguousarray(hm[0, sl].astype(bf16).T)
        in_maps.append({
            "xh": np.ascontiguousarray(np.concatenate([xTc, hTc], axis=1)),
            "cT": np.ascontiguousarray(hm[1, sl].astype(bf16).T),
            "W": W, "U": U, "b": bcat,
        })

    res = run_bass_kernel_spmd(nc, in_maps, core_ids=list(range(NCORES)),
                               trace=_trace)

    h = np.concatenate(
        [res.results[c]["hT_out"].T.astype(np.float32)
         for c in range(NCORES)], 0)
    cc = np.concatenate(
        [res.results[c]["cT_out"].T.astype(np.float32)
         for c in range(NCORES)], 0)
    out = np.stack([h, cc])
    if _return_timing:
        return out, res
    return out
